# revision 13
# baseline (speedup 1.0000x reference)
"""Trainium2 Bass kernel for mutual-nearest-neighbor matching (Lowe ratio test).

Per-core layout: batch b=8 is sharded 1 batch element per NeuronCore (8 cores).
Each core computes, for its batch element:
  sim = d0^T @ d1          [n=4096, m=4096]   (bf16 matmuls, fp32 PSUM accum)
  top-2 + argmax along m  -> matches0 candidates + ratio mask + scores
  sim^T = d1^T @ d0        (second matmul direction)
  top-2 + argmax along n  -> matches1 candidates + ratio mask
  mutual check (fully local, via small gather)
Outputs: matches int32 [4096], scores f32 [4096] per core; host stacks to [8, 4096].

Top-2/argmax strategy per 128x2048 PSUM half-tile:
  ACT evicts PSUM fp32 -> SBUF bf16 (X).
  DVE folds X with 4 levels of pairwise max (2048->128); each final position p
  holds max over the comb group {p + 128*j, j=0..15}.
  DVE Max8 gives the top-8 fold-group maxima (exact top-1, plus the best
  runner-up group max); MaxIndex gives p of the winner.
  GpSimd gathers the winner's 16-candidate comb group from X.
  Batched epilogue: second max = max(runner-up group max, 2nd max within the
  winning group) -- exact; argmax offset recovered via equality match against
  the gathered candidates.  Ratio test + mutual check on [128, 32] tiles.
"""

import sys

if "/opt/trn_rl_repo" not in sys.path:
    sys.path.insert(0, "/opt/trn_rl_repo")

import numpy as np
import ml_dtypes

B, D, N, M = 8, 256, 4096, 4096
NT = N // 128            # 32 row tiles per direction
HALF = M // 2            # 2048 columns per PSUM half-tile
NBANK = HALF // 512      # 4 matmul banks per half-tile
NFOLD = 4                # pairwise-max fold levels per half (2048 -> 128)
FW = HALF >> NFOLD       # 128: final fold width (= comb stride)
NCAND = 1 << NFOLD       # 16 candidates in each comb group
NSLOT = 2 * NT           # 64 (t, h) half-slots per direction
NEG = -1.0e30
RATIO2 = 0.8 * 0.8       # Lowe ratio threshold squared

_CACHE: dict = {}


def _build_program(debug=False):
    import concourse.mybir as mybir
    import concourse.tile as tile
    from concourse import bacc

    dt = mybir.dt
    Alu = mybir.AluOpType

    nc = bacc.Bacc("TRN2", target_bir_lowering=False, debug=False)

    d0_dram = nc.dram_tensor("d0", [2, 128, N], dt.bfloat16, kind="ExternalInput")
    d1_dram = nc.dram_tensor("d1", [2, 128, M], dt.bfloat16, kind="ExternalInput")
    matches_dram = nc.dram_tensor("matches", [N], dt.int32, kind="ExternalOutput")
    scores_dram = nc.dram_tensor("scores", [N], dt.float32, kind="ExternalOutput")
    m1_bounce = nc.dram_tensor("m1_bounce", [M], dt.float32)  # internal
    c_iota16_dram = nc.dram_tensor("c_iota16", [128, NCAND], dt.uint16, kind="ExternalInput")
    c_iotaoff_dram = nc.dram_tensor("c_iotaoff", [128, NSLOT * NCAND], dt.float32, kind="ExternalInput")
    c_hoff_dram = nc.dram_tensor("c_hoff", [128, NSLOT], dt.float32, kind="ExternalInput")
    c_indsn_dram = nc.dram_tensor("c_indsn", [128, NT], dt.float32, kind="ExternalInput")
    c_diagbf_dram = nc.dram_tensor("c_diagbf", [128, 16 * NCAND], dt.bfloat16, kind="ExternalInput")
    c_diagf_dram = nc.dram_tensor("c_diagf", [128, 16 * NT], dt.float32, kind="ExternalInput")
    if debug:
        dbg_m0 = nc.dram_tensor("dbg_m0", [N], dt.float32, kind="ExternalOutput")
        dbg_m1 = nc.dram_tensor("dbg_m1", [M], dt.float32, kind="ExternalOutput")
        dbg_loop = nc.dram_tensor("dbg_loop", [N], dt.float32, kind="ExternalOutput")
        dbg_inds = nc.dram_tensor("dbg_inds", [N], dt.float32, kind="ExternalOutput")

    with tile.TileContext(nc) as tc:
        with (
            tc.tile_pool(name="w", bufs=1) as wpool,
            tc.tile_pool(name="consts", bufs=1) as cpool,
            tc.tile_pool(name="acc", bufs=1) as apool,
            tc.tile_pool(name="x", bufs=8) as xpool,
            tc.tile_pool(name="f", bufs=4) as fpool,
            tc.tile_pool(name="psum", bufs=2, space="PSUM") as ppool,
        ):
            # ---- load descriptors (already bf16, k-major [2, 128, N]) ----
            d0_sb = [wpool.tile([128, N], dt.bfloat16, name=f"d0_{k}", tag=f"d0_{k}") for k in range(2)]
            d1_sb = [wpool.tile([128, M], dt.bfloat16, name=f"d1_{k}", tag=f"d1_{k}") for k in range(2)]
            for k in range(2):
                nc.sync.dma_start(d0_sb[k][:], d0_dram[k])
                nc.sync.dma_start(d1_sb[k][:], d1_dram[k])

            # ---- constants (host-provided) ----
            iota16 = cpool.tile([128, NCAND], dt.uint16, name="iota16", tag="iota16")
            nc.sync.dma_start(iota16[:], c_iota16_dram[:])
            iotaoff = cpool.tile([128, NSLOT * NCAND], dt.float32, name="iotaoff", tag="iotaoff")
            nc.sync.dma_start(iotaoff[:], c_iotaoff_dram[:])
            hoff = cpool.tile([128, NSLOT], dt.float32, name="hoff", tag="hoff")
            nc.sync.dma_start(hoff[:], c_hoff_dram[:])
            indsn = cpool.tile([128, NT], dt.float32, name="indsn", tag="indsn")
            nc.sync.dma_start(indsn[:], c_indsn_dram[:])
            diag_bf = cpool.tile([128, 16 * NCAND], dt.bfloat16, name="diag_bf", tag="diag_bf")
            nc.sync.dma_start(diag_bf[:], c_diagbf_dram[:])
            diag_f = cpool.tile([128, 16 * NT], dt.float32, name="diag_f", tag="diag_f")
            nc.sync.dma_start(diag_f[:], c_diagf_dram[:])

            # ---- per-direction accumulators ----
            t8_acc = [apool.tile([128, NSLOT * 8], dt.bfloat16, name=f"t8_{d}", tag=f"t8_{d}") for d in range(2)]
            pi_acc = [apool.tile([128, NSLOT * 8], dt.uint16, name=f"pi_{d}", tag=f"pi_{d}") for d in range(2)]
            cd_acc = [apool.tile([128, NSLOT * NCAND], dt.float32, name=f"cd_{d}", tag=f"cd_{d}") for d in range(2)]
            gc_acc = apool.tile([128, NSLOT * 16 * NCAND], dt.bfloat16, name="gc_acc", tag="gc_acc")

            # per-direction epilogue results
            m_dir = [apool.tile([128, NT], dt.float32, name=f"mdir_{d}", tag=f"mdir_{d}") for d in range(2)]
            scores0 = apool.tile([128, NT], dt.float32, name="scores0", tag="scores0")

            for dire in range(2):
                lhs = d0_sb if dire == 0 else d1_sb
                rhs = d1_sb if dire == 0 else d0_sb
                t8a, pia, cda = t8_acc[dire], pi_acc[dire], cd_acc[dire]

                for h in range(2):
                    for t in range(NT):
                        s = NT * h + t
                        P = ppool.tile([128, HALF], dt.float32, name="P", tag="P")
                        for k in range(2):
                            for b in range(NBANK):
                                nc.tensor.matmul(
                                    P[:, 512 * b : 512 * (b + 1)],
                                    lhs[k][:, 128 * t : 128 * (t + 1)],
                                    rhs[k][:, HALF * h + 512 * b : HALF * h + 512 * (b + 1)],
                                    start=(k == 0),
                                    stop=(k == 1),
                                )
                        X = xpool.tile([128, HALF], dt.bfloat16, name="X", tag="X")
                        nc.scalar.copy(X[:], P[:])
                        F1 = fpool.tile([128, HALF // 2], dt.bfloat16, name="F1", tag="F1")
                        nc.vector.tensor_max(F1[:], X[:, : HALF // 2], X[:, HALF // 2 :])
                        F2 = fpool.tile([128, HALF // 4], dt.bfloat16, name="F2", tag="F2")
                        nc.vector.tensor_max(F2[:], F1[:, : HALF // 4], F1[:, HALF // 4 :])
                        F3 = fpool.tile([128, HALF // 8], dt.bfloat16, name="F3", tag="F3")
                        nc.vector.tensor_max(F3[:], F2[:, : HALF // 8], F2[:, HALF // 8 :])
                        F4 = fpool.tile([128, FW], dt.bfloat16, name="F4", tag="F4")
                        nc.vector.tensor_max(F4[:], F3[:, :FW], F3[:, FW:])

                        t8_slot = t8a[:, 8 * s : 8 * s + 8]
                        pi_slot = pia[:, 8 * s : 8 * s + 8]
                        nc.vector.max(t8_slot, F4[:])
                        nc.vector.max_index(pi_slot, t8_slot, F4[:])

                        p1f = fpool.tile([128, 1], dt.float32, name="p1f", tag="p1f")
                        nc.gpsimd.tensor_copy(p1f[:], pia[:, 8 * s : 8 * s + 1])
                        ci = fpool.tile([128, NCAND], dt.uint16, name="ci", tag="ci")
                        nc.gpsimd.tensor_scalar(
                            ci[:], iota16[:], p1f[:], None, op0=Alu.add
                        )
                        nc.gpsimd.indirect_copy(
                            gc_acc[:, 16 * NCAND * s : 16 * NCAND * (s + 1)],
                            X[:],
                            ci[:],
                            True,
                        )

                # ---- batched epilogue for this direction ----
                # extract each partition's own candidates from the wrapped gather
                CH = 16  # slots per extraction chunk
                diag_b = diag_bf[:].rearrange("p (a b) -> p a b", a=1).to_broadcast(
                    [128, CH, 16 * NCAND]
                )
                for cch in range(NSLOT // CH):
                    gsl = slice(CH * 16 * NCAND * cch, CH * 16 * NCAND * (cch + 1))
                    gcm = apool.tile(
                        [128, CH * 16 * NCAND], dt.bfloat16,
                        name=f"gcm_{dire}_{cch}", tag="gcm",
                    )
                    nc.vector.tensor_mul(
                        gcm[:].rearrange("p (g e) -> p g e", e=16 * NCAND),
                        gc_acc[:, gsl].rearrange("p (g e) -> p g e", e=16 * NCAND),
                        diag_b,
                    )
                    nc.vector.tensor_reduce(
                        cda[:, CH * NCAND * cch : CH * NCAND * (cch + 1)],
                        gcm[:].rearrange("p (gj u) -> p gj u", u=16),
                        axis=mybir.AxisListType.X,
                        op=Alu.add,
                    )
                A3 = t8a[:].rearrange("p (g e) -> p g e", e=8)
                P3 = pia[:].rearrange("p (g e) -> p g e", e=8)
                C3 = cda[:].rearrange("p (g e) -> p g e", e=NCAND)

                v1b = apool.tile([128, NSLOT], dt.bfloat16, name="v1b", tag="v1b")
                nc.vector.tensor_copy(v1b[:], A3[:, :, 0])
                v1f = apool.tile([128, NSLOT], dt.float32, name="v1f", tag="v1f")
                nc.vector.tensor_copy(v1f[:], v1b[:])
                v2f = apool.tile([128, NSLOT], dt.float32, name="v2f", tag="v2f")
                nc.vector.tensor_copy(v2f[:], A3[:, :, 1])
                pf = apool.tile([128, NSLOT], dt.float32, name="pf", tag="pf")
                nc.vector.tensor_copy(pf[:], P3[:, :, 0])

                # candidate-group analysis
                eq = apool.tile([128, NSLOT * NCAND], dt.float32, name="eq", tag="eq")
                eq3 = eq[:].rearrange("p (g e) -> p g e", e=NCAND)
                v1f3 = v1f[:].to_broadcast([128, NSLOT, NCAND])
                nc.vector.tensor_tensor(eq3, C3, v1f3, op=Alu.is_equal)
                msk = apool.tile([128, NSLOT * NCAND], dt.float32, name="msk", tag="msk")
                msk3 = msk[:].rearrange("p (g e) -> p g e", e=NCAND)
                nc.vector.scalar_tensor_tensor(
                    msk3, eq3, NEG, C3, op0=Alu.mult, op1=Alu.add
                )
                c2 = apool.tile([128, NSLOT], dt.float32, name="c2", tag="c2")
                nc.vector.tensor_reduce(
                    c2[:], msk3, axis=mybir.AxisListType.X, op=Alu.max
                )
                tpd = apool.tile([128, NSLOT * NCAND], dt.float32, name="tpd", tag="tpd")
                nc.vector.tensor_mul(tpd[:], eq[:], iotaoff[:])
                toff = apool.tile([128, NSLOT], dt.float32, name="toff", tag="toff")
                nc.vector.tensor_reduce(
                    toff[:],
                    tpd[:].rearrange("p (g e) -> p g e", e=NCAND),
                    axis=mybir.AxisListType.X,
                    op=Alu.add,
                )
                mabs = apool.tile([128, NSLOT], dt.float32, name="mabs", tag="mabs")
                nc.vector.tensor_add(mabs[:], pf[:], toff[:])
                nc.vector.tensor_add(mabs[:], mabs[:], hoff[:])
                v2in = apool.tile([128, NSLOT], dt.float32, name="v2in", tag="v2in")
                nc.vector.tensor_max(v2in[:], v2f[:], c2[:])

                # combine the two m-halves (slot t vs slot NT+t pair per row)
                lo = slice(0, NT)
                hi = slice(NT, NSLOT)
                is1 = apool.tile([128, NT], dt.uint8, name="is1", tag="is1")
                nc.vector.tensor_tensor(is1[:], v1f[:, hi], v1f[:, lo], op=Alu.is_gt)
                v1g = apool.tile([128, NT], dt.float32, name="v1g", tag="v1g")
                nc.vector.tensor_max(v1g[:], v1f[:, lo], v1f[:, hi])
                v2w = apool.tile([128, NT], dt.float32, name="v2w", tag="v2w")
                nc.vector.tensor_copy(v2w[:], v2in[:, lo])
                nc.vector.copy_predicated(v2w[:], is1[:], v2in[:, hi])
                v1l = apool.tile([128, NT], dt.float32, name="v1l", tag="v1l")
                nc.vector.tensor_copy(v1l[:], v1f[:, hi])
                nc.vector.copy_predicated(v1l[:], is1[:], v1f[:, lo])
                v2g = apool.tile([128, NT], dt.float32, name="v2g", tag="v2g")
                nc.vector.tensor_max(v2g[:], v2w[:], v1l[:])
                mst = apool.tile([128, NT], dt.float32, name="mst", tag="mst")
                nc.vector.tensor_copy(mst[:], mabs[:, lo])
                nc.vector.copy_predicated(mst[:], is1[:], mabs[:, hi])

                # ratio test: dist1 <= r^2 * dist2  <=>  v1 - r^2*v2 >= 1 - r^2
                acc1 = apool.tile([128, NT], dt.float32, name="acc1", tag="acc1")
                nc.vector.scalar_tensor_tensor(
                    acc1[:], v2g[:], -RATIO2, v1g[:], op0=Alu.mult, op1=Alu.add
                )
                maskf = apool.tile([128, NT], dt.uint8, name="maskf", tag="maskf")
                nc.vector.tensor_scalar(
                    maskf[:], acc1[:], 1.0 - RATIO2, None, op0=Alu.is_ge
                )
                if dire == 0:
                    sc = apool.tile([128, NT], dt.float32, name="sc", tag="sc")
                    nc.vector.tensor_scalar(
                        sc[:], v1g[:], 0.5, 0.5, op0=Alu.mult, op1=Alu.add
                    )
                    nc.vector.tensor_mul(scores0[:], sc[:], maskf[:])
                nc.vector.memset(m_dir[dire][:], -1.0)
                nc.vector.copy_predicated(m_dir[dire][:], maskf[:], mst[:])

            # ---- mutual check ----
            # matches1 [128, NT] -> DRAM flat [M] (index m = 128*t + r) -> replicate
            m1_flat_ap = m1_bounce[:].rearrange("(t r) -> r t", r=128)
            nc.sync.dma_start(m1_flat_ap, m_dir[1][:])
            m1_rep = apool.tile([128, M], dt.float32, name="m1_rep", tag="m1_rep")
            nc.sync.dma_start(m1_rep[:1, :], m1_bounce[:][None, :])
            nc.gpsimd.partition_broadcast(m1_rep[:, :], m1_rep[:1, :])

            safe = apool.tile([128, NT], dt.float32, name="safe", tag="safe")
            nc.vector.tensor_scalar_max(safe[:], m_dir[0][:], 0.0)
            safe16 = apool.tile([128, NT], dt.uint16, name="safe16", tag="safe16")
            nc.vector.tensor_copy(safe16[:], safe[:])
            gm = apool.tile([128, 16 * NT], dt.float32, name="gm", tag="gm")
            nc.gpsimd.indirect_copy(gm[:], m1_rep[:], safe16[:], True)
            gmp = apool.tile([128, 16 * NT], dt.float32, name="gmp", tag="gmp")
            nc.vector.tensor_mul(gmp[:], gm[:], diag_f[:])
            loop = apool.tile([128, NT], dt.float32, name="loop", tag="loop")
            nc.vector.tensor_reduce(
                loop[:],
                gmp[:].rearrange("p (j u) -> p j u", u=16),
                axis=mybir.AxisListType.X,
                op=Alu.add,
            )

            g1 = apool.tile([128, NT], dt.uint8, name="g1", tag="g1")
            nc.vector.tensor_scalar(g1[:], m_dir[0][:], -0.5, None, op0=Alu.is_gt)
            g2 = apool.tile([128, NT], dt.uint8, name="g2", tag="g2")
            nc.vector.tensor_tensor(g2[:], indsn[:], loop[:], op=Alu.is_equal)
            okm = apool.tile([128, NT], dt.uint8, name="okm", tag="okm")
            nc.vector.tensor_mul(okm[:], g1[:], g2[:])

            mfin = apool.tile([128, NT], dt.float32, name="mfin", tag="mfin")
            nc.vector.memset(mfin[:], -1.0)
            nc.vector.copy_predicated(mfin[:], okm[:], m_dir[0][:])
            mi32 = apool.tile([128, NT], dt.int32, name="mi32", tag="mi32")
            nc.vector.tensor_copy(mi32[:], mfin[:])

            nc.sync.dma_start(matches_dram[:].rearrange("(t r) -> r t", r=128), mi32[:])
            nc.sync.dma_start(scores_dram[:].rearrange("(t r) -> r t", r=128), scores0[:])
            if debug:
                nc.sync.dma_start(dbg_m0[:].rearrange("(t r) -> r t", r=128), m_dir[0][:])
                nc.sync.dma_start(dbg_m1[:].rearrange("(t r) -> r t", r=128), m_dir[1][:])
                nc.sync.dma_start(dbg_loop[:].rearrange("(t r) -> r t", r=128), loop[:])
                nc.sync.dma_start(dbg_inds[:].rearrange("(t r) -> r t", r=128), indsn[:])

    nc.compile()
    return nc


def _get_program():
    if "nc" not in _CACHE:
        _CACHE["nc"] = _build_program()
    return _CACHE["nc"]


def _make_consts():
    if "consts" in _CACHE:
        return _CACHE["consts"]
    p = np.arange(128)
    j16 = np.arange(16)
    c_iota16 = np.broadcast_to((FW * j16).astype(np.uint16), (128, NCAND)).copy()
    io = FW * (np.arange(NSLOT * NCAND) % NCAND)
    c_iotaoff = np.broadcast_to(io.astype(np.float32), (128, NSLOT * NCAND)).copy()
    c_hoff = np.zeros((128, NSLOT), np.float32)
    c_hoff[:, NT:] = float(HALF)
    c_indsn = (128 * np.arange(NT)[None, :] + p[:, None]).astype(np.float32)
    diag = (np.arange(16)[None, :] == (p % 16)[:, None])  # [128, 16]
    c_diagbf = np.tile(diag, (1, NCAND)).astype(ml_dtypes.bfloat16)
    c_diagf = np.tile(diag, (1, NT)).astype(np.float32)
    consts = {
        "c_iota16": c_iota16,
        "c_iotaoff": c_iotaoff,
        "c_hoff": c_hoff,
        "c_indsn": c_indsn,
        "c_diagbf": c_diagbf,
        "c_diagf": c_diagf,
    }
    _CACHE["consts"] = consts
    return consts


def _make_in_maps(descriptors0, descriptors1):
    consts = _make_consts()
    in_maps = []
    for c in range(B):
        a = np.ascontiguousarray(descriptors0[c].reshape(2, 128, N)).astype(
            ml_dtypes.bfloat16
        )
        bb = np.ascontiguousarray(descriptors1[c].reshape(2, 128, M)).astype(
            ml_dtypes.bfloat16
        )
        in_maps.append({"d0": a, "d1": bb, **consts})
    return in_maps


def kernel(descriptors0: np.ndarray, descriptors1: np.ndarray):
    from concourse.bass_utils import run_bass_kernel_spmd

    nc = _get_program()
    in_maps = _make_in_maps(descriptors0, descriptors1)
    res = run_bass_kernel_spmd(nc, in_maps, core_ids=list(range(B)))
    matches = np.stack([np.asarray(res.results[c]["matches"]) for c in range(B)])
    scores = np.stack([np.asarray(res.results[c]["scores"]) for c in range(B)])
    return matches.astype(np.int32), scores.astype(np.float32)


# revision 14
# speedup vs baseline: 1.0644x; 1.0644x over previous
"""Trainium2 Bass kernel for mutual-nearest-neighbor matching (Lowe ratio test).

Per-core layout: batch b=8 is sharded 1 batch element per NeuronCore (8 cores).
Each core computes, for its batch element:
  sim = d0^T @ d1          [n=4096, m=4096]   (bf16 matmuls, fp32 PSUM accum)
  top-2 + argmax along m  -> matches0 candidates + ratio mask + scores
  sim^T = d1^T @ d0        (second matmul direction)
  top-2 + argmax along n  -> matches1 candidates + ratio mask
  mutual check (fully local, via small gather)
Outputs: matches int32 [4096], scores f32 [4096] per core; host stacks to [8, 4096].

Top-2/argmax strategy per 128x2048 PSUM half-tile:
  ACT evicts PSUM fp32 -> SBUF bf16 (X).
  DVE folds X with 4 levels of pairwise max (2048->128); each final position p
  holds max over the comb group {p + 128*j, j=0..15}.
  DVE Max8 gives the top-8 fold-group maxima (exact top-1, plus the best
  runner-up group max); MaxIndex gives p of the winner.
  GpSimd gathers the winner's 16-candidate comb group from X.
  Batched epilogue: second max = max(runner-up group max, 2nd max within the
  winning group) -- exact; argmax offset recovered via equality match against
  the gathered candidates.  Ratio test + mutual check on [128, 32] tiles.
"""

import sys

if "/opt/trn_rl_repo" not in sys.path:
    sys.path.insert(0, "/opt/trn_rl_repo")

import numpy as np
import ml_dtypes

B, D, N, M = 8, 256, 4096, 4096
NT = N // 128            # 32 row tiles per direction
HALF = M // 2            # 2048 columns per PSUM half-tile
NBANK = HALF // 512      # 4 matmul banks per half-tile
NFOLD = 4                # pairwise-max fold levels per half (2048 -> 128)
FW = HALF >> NFOLD       # 128: final fold width (= comb stride)
NCAND = 1 << NFOLD       # 16 candidates in each comb group
NSLOT = 2 * NT           # 64 (t, h) half-slots per direction
NEG = -1.0e30
RATIO2 = 0.8 * 0.8       # Lowe ratio threshold squared

_CACHE: dict = {}


def _build_program(debug=False):
    import concourse.mybir as mybir
    import concourse.tile as tile
    from concourse import bacc

    dt = mybir.dt
    Alu = mybir.AluOpType

    nc = bacc.Bacc("TRN2", target_bir_lowering=False, debug=False)

    d0_dram = nc.dram_tensor("d0", [2, 128, N], dt.bfloat16, kind="ExternalInput")
    d1_dram = nc.dram_tensor("d1", [2, 128, M], dt.bfloat16, kind="ExternalInput")
    matches_dram = nc.dram_tensor("matches", [N], dt.int32, kind="ExternalOutput")
    scores_dram = nc.dram_tensor("scores", [N], dt.float32, kind="ExternalOutput")
    m1_bounce = nc.dram_tensor("m1_bounce", [M], dt.float32)  # internal
    c_iota16_dram = nc.dram_tensor("c_iota16", [128, NCAND], dt.uint16, kind="ExternalInput")
    c_iotaoff_dram = nc.dram_tensor("c_iotaoff", [128, NSLOT * NCAND], dt.float32, kind="ExternalInput")
    c_hoff_dram = nc.dram_tensor("c_hoff", [128, NSLOT], dt.float32, kind="ExternalInput")
    c_indsn_dram = nc.dram_tensor("c_indsn", [128, NT], dt.float32, kind="ExternalInput")
    c_diagbf_dram = nc.dram_tensor("c_diagbf", [128, 16 * NCAND], dt.bfloat16, kind="ExternalInput")
    c_diagf_dram = nc.dram_tensor("c_diagf", [128, 16 * NT], dt.float32, kind="ExternalInput")
    if debug:
        dbg_m0 = nc.dram_tensor("dbg_m0", [N], dt.float32, kind="ExternalOutput")
        dbg_m1 = nc.dram_tensor("dbg_m1", [M], dt.float32, kind="ExternalOutput")
        dbg_loop = nc.dram_tensor("dbg_loop", [N], dt.float32, kind="ExternalOutput")
        dbg_inds = nc.dram_tensor("dbg_inds", [N], dt.float32, kind="ExternalOutput")

    with tile.TileContext(nc) as tc:
        with (
            tc.tile_pool(name="w", bufs=1) as wpool,
            tc.tile_pool(name="consts", bufs=1) as cpool,
            tc.tile_pool(name="acc", bufs=1) as apool,
            tc.tile_pool(name="x", bufs=8) as xpool,
            tc.tile_pool(name="f", bufs=4) as fpool,
            tc.tile_pool(name="psum", bufs=2, space="PSUM") as ppool,
        ):
            # ---- load descriptors (already bf16, k-major [2, 128, N]) ----
            d0_sb = [wpool.tile([128, N], dt.bfloat16, name=f"d0_{k}", tag=f"d0_{k}") for k in range(2)]
            d1_sb = [wpool.tile([128, M], dt.bfloat16, name=f"d1_{k}", tag=f"d1_{k}") for k in range(2)]
            for k in range(2):
                nc.sync.dma_start(d0_sb[k][:], d0_dram[k])
                nc.sync.dma_start(d1_sb[k][:], d1_dram[k])

            # ---- constants (host-provided) ----
            iota16 = cpool.tile([128, NCAND], dt.uint16, name="iota16", tag="iota16")
            nc.sync.dma_start(iota16[:], c_iota16_dram[:])
            iotaoff = cpool.tile([128, NSLOT * NCAND], dt.float32, name="iotaoff", tag="iotaoff")
            nc.sync.dma_start(iotaoff[:], c_iotaoff_dram[:])
            hoff = cpool.tile([128, NSLOT], dt.float32, name="hoff", tag="hoff")
            nc.sync.dma_start(hoff[:], c_hoff_dram[:])
            indsn = cpool.tile([128, NT], dt.float32, name="indsn", tag="indsn")
            nc.sync.dma_start(indsn[:], c_indsn_dram[:])
            diag_bf = cpool.tile([128, 16 * NCAND], dt.bfloat16, name="diag_bf", tag="diag_bf")
            nc.sync.dma_start(diag_bf[:], c_diagbf_dram[:])
            diag_f = cpool.tile([128, 16 * NT], dt.float32, name="diag_f", tag="diag_f")
            nc.sync.dma_start(diag_f[:], c_diagf_dram[:])

            # ---- per-direction accumulators ----
            t8_acc = [apool.tile([128, NSLOT * 8], dt.bfloat16, name=f"t8_{d}", tag=f"t8_{d}") for d in range(2)]
            pi_acc = [apool.tile([128, NSLOT * 8], dt.uint16, name=f"pi_{d}", tag=f"pi_{d}") for d in range(2)]
            cd_acc = [apool.tile([128, NSLOT * NCAND], dt.float32, name=f"cd_{d}", tag=f"cd_{d}") for d in range(2)]
            gc_acc = apool.tile([128, NSLOT * 16 * NCAND], dt.bfloat16, name="gc_acc", tag="gc_acc")

            # per-direction epilogue results
            m_dir = [apool.tile([128, NT], dt.float32, name=f"mdir_{d}", tag=f"mdir_{d}") for d in range(2)]
            scores0 = apool.tile([128, NT], dt.float32, name="scores0", tag="scores0")

            for dire in range(2):
                lhs = d0_sb if dire == 0 else d1_sb
                rhs = d1_sb if dire == 0 else d0_sb
                t8a, pia, cda = t8_acc[dire], pi_acc[dire], cd_acc[dire]

                # software-pipelined emission: mm(T) | evict(T-1) | dve(T-2) | gather(T-3)
                SK_E, SK_D, SK_G = 1, 2, 3
                items = [(h, t) for h in range(2) for t in range(NT)]
                n_items = len(items)
                P_q = {}
                X_q = {}
                CI_q = {}

                def emit_mm(i):
                    h, t = items[i]
                    P = ppool.tile([128, HALF], dt.float32, name=f"P_{dire}_{i}", tag="P")
                    P_q[i] = P
                    for k in range(2):
                        for b in range(NBANK):
                            nc.tensor.matmul(
                                P[:, 512 * b : 512 * (b + 1)],
                                lhs[k][:, 128 * t : 128 * (t + 1)],
                                rhs[k][:, HALF * h + 512 * b : HALF * h + 512 * (b + 1)],
                                start=(k == 0),
                                stop=(k == 1),
                            )

                def emit_evict(i):
                    P = P_q.pop(i)
                    X = xpool.tile([128, HALF], dt.bfloat16, name=f"X_{dire}_{i}", tag="X")
                    X_q[i] = X
                    nc.scalar.copy(X[:], P[:])

                def emit_dve(i):
                    h, t = items[i]
                    s = NT * h + t
                    X = X_q[i]
                    F1 = fpool.tile([128, HALF // 2], dt.bfloat16, name=f"F1_{dire}_{i}", tag="F1")
                    nc.vector.tensor_max(F1[:], X[:, : HALF // 2], X[:, HALF // 2 :])
                    F2 = fpool.tile([128, HALF // 4], dt.bfloat16, name=f"F2_{dire}_{i}", tag="F2")
                    nc.vector.tensor_max(F2[:], F1[:, : HALF // 4], F1[:, HALF // 4 :])
                    F3 = fpool.tile([128, HALF // 8], dt.bfloat16, name=f"F3_{dire}_{i}", tag="F3")
                    nc.vector.tensor_max(F3[:], F2[:, : HALF // 8], F2[:, HALF // 8 :])
                    F4 = fpool.tile([128, FW], dt.bfloat16, name=f"F4_{dire}_{i}", tag="F4")
                    nc.vector.tensor_max(F4[:], F3[:, :FW], F3[:, FW:])
                    t8_slot = t8a[:, 8 * s : 8 * s + 8]
                    pi_slot = pia[:, 8 * s : 8 * s + 8]
                    nc.vector.max(t8_slot, F4[:])
                    nc.vector.max_index(pi_slot, t8_slot, F4[:])
                    p1f = fpool.tile([128, 1], dt.float32, name=f"p1f_{dire}_{i}", tag="p1f")
                    nc.vector.tensor_copy(p1f[:], pia[:, 8 * s : 8 * s + 1])
                    ci = fpool.tile([128, NCAND], dt.uint16, name=f"ci_{dire}_{i}", tag="ci")
                    nc.vector.tensor_scalar(ci[:], iota16[:], p1f[:], None, op0=Alu.add)
                    CI_q[i] = ci

                def emit_gather(i):
                    h, t = items[i]
                    s = NT * h + t
                    X = X_q.pop(i)
                    ci = CI_q.pop(i)
                    nc.gpsimd.indirect_copy(
                        gc_acc[:, 16 * NCAND * s : 16 * NCAND * (s + 1)],
                        X[:],
                        ci[:],
                        True,
                    )

                for step in range(n_items + SK_G):
                    if step < n_items:
                        emit_mm(step)
                    if SK_E <= step < n_items + SK_E:
                        emit_evict(step - SK_E)
                    if SK_D <= step < n_items + SK_D:
                        emit_dve(step - SK_D)
                    if SK_G <= step < n_items + SK_G:
                        emit_gather(step - SK_G)

                # ---- batched epilogue for this direction ----
                # extract each partition's own candidates from the wrapped gather
                CH = 16  # slots per extraction chunk
                diag_b = diag_bf[:].rearrange("p (a b) -> p a b", a=1).to_broadcast(
                    [128, CH, 16 * NCAND]
                )
                for cch in range(NSLOT // CH):
                    gsl = slice(CH * 16 * NCAND * cch, CH * 16 * NCAND * (cch + 1))
                    gcm = apool.tile(
                        [128, CH * 16 * NCAND], dt.bfloat16,
                        name=f"gcm_{dire}_{cch}", tag="gcm",
                    )
                    nc.vector.tensor_mul(
                        gcm[:].rearrange("p (g e) -> p g e", e=16 * NCAND),
                        gc_acc[:, gsl].rearrange("p (g e) -> p g e", e=16 * NCAND),
                        diag_b,
                    )
                    nc.vector.tensor_reduce(
                        cda[:, CH * NCAND * cch : CH * NCAND * (cch + 1)],
                        gcm[:].rearrange("p (gj u) -> p gj u", u=16),
                        axis=mybir.AxisListType.X,
                        op=Alu.add,
                    )
                A3 = t8a[:].rearrange("p (g e) -> p g e", e=8)
                P3 = pia[:].rearrange("p (g e) -> p g e", e=8)
                C3 = cda[:].rearrange("p (g e) -> p g e", e=NCAND)

                v1b = apool.tile([128, NSLOT], dt.bfloat16, name="v1b", tag="v1b")
                nc.vector.tensor_copy(v1b[:], A3[:, :, 0])
                v1f = apool.tile([128, NSLOT], dt.float32, name="v1f", tag="v1f")
                nc.vector.tensor_copy(v1f[:], v1b[:])
                v2f = apool.tile([128, NSLOT], dt.float32, name="v2f", tag="v2f")
                nc.vector.tensor_copy(v2f[:], A3[:, :, 1])
                pf = apool.tile([128, NSLOT], dt.float32, name="pf", tag="pf")
                nc.vector.tensor_copy(pf[:], P3[:, :, 0])

                # candidate-group analysis
                eq = apool.tile([128, NSLOT * NCAND], dt.float32, name="eq", tag="eq")
                eq3 = eq[:].rearrange("p (g e) -> p g e", e=NCAND)
                v1f3 = v1f[:].to_broadcast([128, NSLOT, NCAND])
                nc.vector.tensor_tensor(eq3, C3, v1f3, op=Alu.is_equal)
                msk = apool.tile([128, NSLOT * NCAND], dt.float32, name="msk", tag="msk")
                msk3 = msk[:].rearrange("p (g e) -> p g e", e=NCAND)
                nc.vector.scalar_tensor_tensor(
                    msk3, eq3, NEG, C3, op0=Alu.mult, op1=Alu.add
                )
                c2 = apool.tile([128, NSLOT], dt.float32, name="c2", tag="c2")
                nc.vector.tensor_reduce(
                    c2[:], msk3, axis=mybir.AxisListType.X, op=Alu.max
                )
                tpd = apool.tile([128, NSLOT * NCAND], dt.float32, name="tpd", tag="tpd")
                nc.vector.tensor_mul(tpd[:], eq[:], iotaoff[:])
                toff = apool.tile([128, NSLOT], dt.float32, name="toff", tag="toff")
                nc.vector.tensor_reduce(
                    toff[:],
                    tpd[:].rearrange("p (g e) -> p g e", e=NCAND),
                    axis=mybir.AxisListType.X,
                    op=Alu.add,
                )
                mabs = apool.tile([128, NSLOT], dt.float32, name="mabs", tag="mabs")
                nc.vector.tensor_add(mabs[:], pf[:], toff[:])
                nc.vector.tensor_add(mabs[:], mabs[:], hoff[:])
                v2in = apool.tile([128, NSLOT], dt.float32, name="v2in", tag="v2in")
                nc.vector.tensor_max(v2in[:], v2f[:], c2[:])

                # combine the two m-halves (slot t vs slot NT+t pair per row)
                lo = slice(0, NT)
                hi = slice(NT, NSLOT)
                is1 = apool.tile([128, NT], dt.uint8, name="is1", tag="is1")
                nc.vector.tensor_tensor(is1[:], v1f[:, hi], v1f[:, lo], op=Alu.is_gt)
                v1g = apool.tile([128, NT], dt.float32, name="v1g", tag="v1g")
                nc.vector.tensor_max(v1g[:], v1f[:, lo], v1f[:, hi])
                v2w = apool.tile([128, NT], dt.float32, name="v2w", tag="v2w")
                nc.vector.tensor_copy(v2w[:], v2in[:, lo])
                nc.vector.copy_predicated(v2w[:], is1[:], v2in[:, hi])
                v1l = apool.tile([128, NT], dt.float32, name="v1l", tag="v1l")
                nc.vector.tensor_copy(v1l[:], v1f[:, hi])
                nc.vector.copy_predicated(v1l[:], is1[:], v1f[:, lo])
                v2g = apool.tile([128, NT], dt.float32, name="v2g", tag="v2g")
                nc.vector.tensor_max(v2g[:], v2w[:], v1l[:])
                mst = apool.tile([128, NT], dt.float32, name="mst", tag="mst")
                nc.vector.tensor_copy(mst[:], mabs[:, lo])
                nc.vector.copy_predicated(mst[:], is1[:], mabs[:, hi])

                # ratio test: dist1 <= r^2 * dist2  <=>  v1 - r^2*v2 >= 1 - r^2
                acc1 = apool.tile([128, NT], dt.float32, name="acc1", tag="acc1")
                nc.vector.scalar_tensor_tensor(
                    acc1[:], v2g[:], -RATIO2, v1g[:], op0=Alu.mult, op1=Alu.add
                )
                maskf = apool.tile([128, NT], dt.uint8, name="maskf", tag="maskf")
                nc.vector.tensor_scalar(
                    maskf[:], acc1[:], 1.0 - RATIO2, None, op0=Alu.is_ge
                )
                if dire == 0:
                    sc = apool.tile([128, NT], dt.float32, name="sc", tag="sc")
                    nc.vector.tensor_scalar(
                        sc[:], v1g[:], 0.5, 0.5, op0=Alu.mult, op1=Alu.add
                    )
                    nc.vector.tensor_mul(scores0[:], sc[:], maskf[:])
                nc.vector.memset(m_dir[dire][:], -1.0)
                nc.vector.copy_predicated(m_dir[dire][:], maskf[:], mst[:])

            # ---- mutual check ----
            # matches1 [128, NT] -> DRAM flat [M] (index m = 128*t + r) -> replicate
            m1_flat_ap = m1_bounce[:].rearrange("(t r) -> r t", r=128)
            nc.sync.dma_start(m1_flat_ap, m_dir[1][:])
            m1_rep = apool.tile([128, M], dt.float32, name="m1_rep", tag="m1_rep")
            nc.sync.dma_start(m1_rep[:1, :], m1_bounce[:][None, :])
            nc.gpsimd.partition_broadcast(m1_rep[:, :], m1_rep[:1, :])

            safe = apool.tile([128, NT], dt.float32, name="safe", tag="safe")
            nc.vector.tensor_scalar_max(safe[:], m_dir[0][:], 0.0)
            safe16 = apool.tile([128, NT], dt.uint16, name="safe16", tag="safe16")
            nc.vector.tensor_copy(safe16[:], safe[:])
            gm = apool.tile([128, 16 * NT], dt.float32, name="gm", tag="gm")
            nc.gpsimd.indirect_copy(gm[:], m1_rep[:], safe16[:], True)
            gmp = apool.tile([128, 16 * NT], dt.float32, name="gmp", tag="gmp")
            nc.vector.tensor_mul(gmp[:], gm[:], diag_f[:])
            loop = apool.tile([128, NT], dt.float32, name="loop", tag="loop")
            nc.vector.tensor_reduce(
                loop[:],
                gmp[:].rearrange("p (j u) -> p j u", u=16),
                axis=mybir.AxisListType.X,
                op=Alu.add,
            )

            g1 = apool.tile([128, NT], dt.uint8, name="g1", tag="g1")
            nc.vector.tensor_scalar(g1[:], m_dir[0][:], -0.5, None, op0=Alu.is_gt)
            g2 = apool.tile([128, NT], dt.uint8, name="g2", tag="g2")
            nc.vector.tensor_tensor(g2[:], indsn[:], loop[:], op=Alu.is_equal)
            okm = apool.tile([128, NT], dt.uint8, name="okm", tag="okm")
            nc.vector.tensor_mul(okm[:], g1[:], g2[:])

            mfin = apool.tile([128, NT], dt.float32, name="mfin", tag="mfin")
            nc.vector.memset(mfin[:], -1.0)
            nc.vector.copy_predicated(mfin[:], okm[:], m_dir[0][:])
            mi32 = apool.tile([128, NT], dt.int32, name="mi32", tag="mi32")
            nc.vector.tensor_copy(mi32[:], mfin[:])

            nc.sync.dma_start(matches_dram[:].rearrange("(t r) -> r t", r=128), mi32[:])
            nc.sync.dma_start(scores_dram[:].rearrange("(t r) -> r t", r=128), scores0[:])
            if debug:
                nc.sync.dma_start(dbg_m0[:].rearrange("(t r) -> r t", r=128), m_dir[0][:])
                nc.sync.dma_start(dbg_m1[:].rearrange("(t r) -> r t", r=128), m_dir[1][:])
                nc.sync.dma_start(dbg_loop[:].rearrange("(t r) -> r t", r=128), loop[:])
                nc.sync.dma_start(dbg_inds[:].rearrange("(t r) -> r t", r=128), indsn[:])

    nc.compile()
    return nc


def _get_program():
    if "nc" not in _CACHE:
        _CACHE["nc"] = _build_program()
    return _CACHE["nc"]


def _make_consts():
    if "consts" in _CACHE:
        return _CACHE["consts"]
    p = np.arange(128)
    j16 = np.arange(16)
    c_iota16 = np.broadcast_to((FW * j16).astype(np.uint16), (128, NCAND)).copy()
    io = FW * (np.arange(NSLOT * NCAND) % NCAND)
    c_iotaoff = np.broadcast_to(io.astype(np.float32), (128, NSLOT * NCAND)).copy()
    c_hoff = np.zeros((128, NSLOT), np.float32)
    c_hoff[:, NT:] = float(HALF)
    c_indsn = (128 * np.arange(NT)[None, :] + p[:, None]).astype(np.float32)
    diag = (np.arange(16)[None, :] == (p % 16)[:, None])  # [128, 16]
    c_diagbf = np.tile(diag, (1, NCAND)).astype(ml_dtypes.bfloat16)
    c_diagf = np.tile(diag, (1, NT)).astype(np.float32)
    consts = {
        "c_iota16": c_iota16,
        "c_iotaoff": c_iotaoff,
        "c_hoff": c_hoff,
        "c_indsn": c_indsn,
        "c_diagbf": c_diagbf,
        "c_diagf": c_diagf,
    }
    _CACHE["consts"] = consts
    return consts


def _make_in_maps(descriptors0, descriptors1):
    consts = _make_consts()
    in_maps = []
    for c in range(B):
        a = np.ascontiguousarray(descriptors0[c].reshape(2, 128, N)).astype(
            ml_dtypes.bfloat16
        )
        bb = np.ascontiguousarray(descriptors1[c].reshape(2, 128, M)).astype(
            ml_dtypes.bfloat16
        )
        in_maps.append({"d0": a, "d1": bb, **consts})
    return in_maps


def kernel(descriptors0: np.ndarray, descriptors1: np.ndarray):
    from concourse.bass_utils import run_bass_kernel_spmd

    nc = _get_program()
    in_maps = _make_in_maps(descriptors0, descriptors1)
    res = run_bass_kernel_spmd(nc, in_maps, core_ids=list(range(B)))
    matches = np.stack([np.asarray(res.results[c]["matches"]) for c in range(B)])
    scores = np.stack([np.asarray(res.results[c]["scores"]) for c in range(B)])
    return matches.astype(np.int32), scores.astype(np.float32)
